# revision 12
# baseline (speedup 1.0000x reference)
"""Bidirectional 2-layer GRU decoder on 8 Trainium2 NeuronCores.

Sharding: cores 0-3 run the forward direction on batch quarters (16 rows
each), cores 4-7 the backward direction (time-reversed input). Each core
runs layer 0 over all 512 steps, then layer 1 (no cross-core traffic).

On-core layout is feature-major ("transposed"): hidden state h^T lives as
[128 partitions, 4 k-chunks, 16 batch] and the recurrent matmul computes
gh^T = Whh @ h^T with the weight chunk stationary ([K=128 of H, M=128 of
3H]) and h^T moving (N=16). Gates come out feature-major, so the GRU
update produces the next h^T directly - no per-step transpose.

The input transform gi^T = Wih @ x^T + bias is batched over 32-step
blocks (moving N=512) and drip-fed between recurrence steps so the PE
fills recurrence tail gaps with gi matmuls.

Matmul operands are fp16 (weights, x, h); accumulation and gate math fp32.
"""

import re

import numpy as np

import concourse.bass as bass
import concourse.mybir as mybir
import concourse.tile as tile
from concourse.bass_utils import run_bass_kernel_spmd

F32 = mybir.dt.float32
F16 = mybir.dt.float16
ALU = mybir.AluOpType
ACTF = mybir.ActivationFunctionType

B, S, D, H, L = 64, 512, 512, 512, 2
NCORES = 8
BC = B // (NCORES // 2)  # 16 batch rows per core
KC = H // 128            # 4 contraction chunks
MC = (3 * H) // 128      # 12 gate-row chunks
TB = 16                  # steps per gi block
G3 = 3 * H

# ---------------------------------------------------------------------------
# Workaround: this walrus accepts only one sync-wait command on a CTRL
# (Drain) instruction, but TileContext's tail drain stacks one wait per used
# DMA queue. Emit standalone wait_ge instructions instead.
_ENGINE_PREFIXES = ("Activation", "PE", "DVE", "Pool", "SP")


def _patched_drain_and_barrier(self, tick_clock, wait_clock):
    ticks = [int(v) for v in re.findall(r"\d+", repr(tick_clock.global_clock))]
    for proc, sem in sorted(self.sems.allocated().items()):
        t = ticks[proc] if proc < len(ticks) else 0
        if t <= 0 or sem.name.startswith(_ENGINE_PREFIXES):
            continue
        mult = 16 if sem.name.startswith("DMA") else 1
        self.nc.sync.wait_ge(sem, t * mult)
    self.nc.sync.drain()
    self.nc.all_engine_barrier()
    assert self.sems is not None
    popped = self.nc._tile_sem_poison_stack.pop()
    assert popped is self._sem_poison
    self.nc.clear_and_free_semaphores(list(self.sems.allocated().values()))
    self.nc.all_engine_barrier()


def _apply_tile_fix():
    tile.TileContext._drain_and_barrier = _patched_drain_and_barrier


def _split_multi_waits(nc):
    """This walrus allows at most one sync-wait command per instruction.
    Tile attaches one wait per producer proc; move extras onto standalone
    no-fuse NoOps placed immediately before the instruction."""
    ctr = [0]
    for blk in nc.m.functions[0].blocks:
        insts = blk.instructions
        out = []
        for inst in insts:
            si = inst.sync_info
            if si is not None and len(si.on_wait) > 1:
                waits = list(si.on_wait)
                for w in waits[:-1]:
                    ctr[0] += 1
                    out.append(mybir.InstNoOp(
                        name=f"splitw-{ctr[0]}",
                        engine=inst.engine,
                        bass_nofuse=True,
                        sync_info=mybir.SyncInfo(on_wait=[w], on_update=[]),
                    ))
                inst.sync_info = mybir.SyncInfo(
                    on_wait=[waits[-1]], on_update=list(si.on_update))
            out.append(inst)
        if len(out) != len(insts):
            blk.instructions = out


# ---------------------------------------------------------------------------


def build_program(s_len=S):
    """Build the per-core Bass program (SPMD: same program on all 8 cores)."""
    _apply_tile_fix()
    nb = s_len // TB

    nc = bass.Bass("TRN2", target_bir_lowering=False, debug=False,
                   num_devices=NCORES)

    # Inputs (per core). All partition-first layouts.
    xT = nc.dram_tensor("xT", [128, KC, s_len * BC], F16, kind="ExternalInput")
    wih = nc.dram_tensor("wih", [L, 128, KC, G3], F16, kind="ExternalInput")
    whh = nc.dram_tensor("whh", [L, 128, KC, G3], F16, kind="ExternalInput")
    gbias = nc.dram_tensor("gbias", [L, 128, MC], F32, kind="ExternalInput")
    ident = nc.dram_tensor("ident", [128, 128], F16, kind="ExternalInput")
    bhhnb = nc.dram_tensor("bhhnb", [L, 128, KC, BC], F16,
                           kind="ExternalInput")
    h0T = nc.dram_tensor("h0T", [L, 128, KC, BC], F32, kind="ExternalInput")

    y_out = nc.dram_tensor("y_out", [nb, 128, KC, TB * BC], F16,
                           kind="ExternalOutput")
    h_out = nc.dram_tensor("h_out", [L, 128, KC, BC], F16,
                           kind="ExternalOutput")

    # gate chunk index groups (PyTorch row order: r, z, n)
    RCH = list(range(0, 4))
    ZCH = list(range(4, 8))
    NCH = list(range(8, 12))

    with tile.TileContext(nc) as tc:
        with (
            tc.tile_pool(name="wpool", bufs=1) as wpool,
            tc.tile_pool(name="state", bufs=1) as state,
            tc.tile_pool(name="gi", bufs=2) as gip,
            tc.tile_pool(name="step", bufs=2) as stp,
            tc.tile_pool(name="psum1", bufs=1, space="PSUM") as psp1,
            tc.tile_pool(name="psum2", bufs=2, space="PSUM") as psp2,
        ):
            out0 = state.tile([128, KC, s_len * BC], F16, tag="out0")
            gb_sb = state.tile([128, L, MC], F32, tag="gb")
            id_sb = state.tile([128, 128], F16, tag="ident")
            bnb_sb = state.tile([128, L, KC, BC], F16, tag="bnb")
            nc.sync.dma_start(gb_sb[:, 0, :], gbias[0])
            nc.sync.dma_start(gb_sb[:, 1, :], gbias[1])
            nc.sync.dma_start(id_sb[:], ident[:])
            nc.sync.dma_start(bnb_sb[:, 0], bhhnb[0])
            nc.sync.dma_start(bnb_sb[:, 1], bhhnb[1])

            wih_sb, whh_sb, hinit = [], [], []
            for l in range(L):
                wi = wpool.tile([128, KC, G3], F16, tag=f"wih{l}")
                wh = wpool.tile([128, KC, G3], F16, tag=f"whh{l}")
                nc.sync.dma_start(wi[:], wih[l])
                nc.sync.dma_start(wh[:], whh[l])
                wih_sb.append(wi)
                whh_sb.append(wh)
                h0_sb = stp.tile([128, KC, BC], F32, tag=f"h0_{l}")
                hi = state.tile([128, KC, BC], F16, tag=f"hinit{l}")
                nc.sync.dma_start(h0_sb[:], h0T[l])
                nc.vector.tensor_copy(out=hi[:], in_=h0_sb[:])
                hinit.append(hi)

            def gi_ops(l, blk):
                c0 = blk * TB * BC
                gi_sb = gip.tile([128, MC, TB * BC], F16, tag=f"gi_sb{l}")
                xin = None
                if l == 0:
                    xin = gip.tile([128, KC, TB * BC], F16, tag="xin")

                def gen():
                    if l == 0:
                        for k in range(KC):
                            yield lambda k=k: nc.sync.dma_start(
                                xin[:, k, :], xT[:, k, c0:c0 + TB * BC])

                    def rhs_ap(k):
                        if l == 0:
                            return xin[:, k, :]
                        return out0[:, k, c0:c0 + TB * BC]

                    def mm(m, k, ps):
                        nc.tensor.matmul(
                            ps[:],
                            wih_sb[l][:, k, 128 * m:128 * (m + 1)],
                            rhs_ap(k),
                            start=(k == 0), stop=(k == KC - 1))

                    def cp(m, ps):
                        nc.scalar.activation(
                            gi_sb[:, m, :], ps[:], ACTF.Identity,
                            bias=gb_sb[:, l, m:m + 1])

                    for m in range(MC):
                        ps_gi = psp2.tile([128, TB * BC], F32, tag="ps_gi")
                        for k in range(KC):
                            yield lambda m=m, k=k, ps=ps_gi: mm(m, k, ps)
                        yield lambda m=m, ps=ps_gi: cp(m, ps)

                return gen(), gi_sb

            class Feeder:
                def __init__(self, l):
                    self.l = l
                    self.gen = None
                    self.out = None

                def start(self, blk):
                    self.gen, self.out = gi_ops(self.l, blk)

                def pump(self, n):
                    for _ in range(n):
                        if self.gen is None:
                            return
                        try:
                            op = next(self.gen)
                        except StopIteration:
                            self.gen = None
                        else:
                            op()

                def finish(self):
                    while self.gen is not None:
                        self.pump(1000)
                    return self.out

            def emit_step(l, blk, tt, gi_cur, y_stage, prev_stage, feeder,
                          pump_n):
                t = blk * TB + tt
                if t == 0:
                    hprev = hinit[l][:]
                elif l == 0:
                    hprev = out0[:, :, (t - 1) * BC:t * BC]
                elif tt == 0:
                    hprev = prev_stage[:, :, (TB - 1) * BC:TB * BC]
                else:
                    hprev = y_stage[:, :, (tt - 1) * BC:tt * BC]

                ps_r = psp1.tile([128, KC, BC], F32, tag=f"ps_r{l}")
                ps_n = psp1.tile([128, KC, BC], F32, tag=f"ps_n{l}")
                ps_z = psp1.tile([128, KC, BC], F32, tag=f"ps_z{l}")

                ts0 = tt * BC

                def gates(chunks, ps, extra):
                    # start=True clears has_written for the WHOLE bank, so
                    # only the very first matmul of the tile may set it.
                    for mi, m in enumerate(chunks):
                        for k in range(KC):
                            nc.tensor.matmul(
                                ps[:, mi, :],
                                whh_sb[l][:, k, 128 * m:128 * (m + 1)],
                                hprev[:, k, :],
                                start=(mi == 0 and k == 0), stop=False,
                                skip_group_check=True)
                    # fold gi (or bhh_n) into the whole gate tile in one MM
                    nc.tensor.matmul(
                        ps[:], id_sb[:], extra,
                        start=False, stop=True, skip_group_check=True)

                gi_r = gi_cur[:, 0:4, ts0:ts0 + BC]
                gi_z = gi_cur[:, 4:8, ts0:ts0 + BC]
                gi_n = gi_cur[:, 8:12, ts0:ts0 + BC]

                gates(RCH, ps_r, gi_r)
                feeder.pump(pump_n)
                gates(NCH, ps_n, bnb_sb[:, l])

                r = stp.tile([128, KC, BC], F32, tag=f"r{l}")
                nc.scalar.activation(r[:], ps_r[:], ACTF.Sigmoid)

                gates(ZCH, ps_z, gi_z)

                nr = stp.tile([128, KC, BC], F16, tag=f"nr{l}")
                nc.vector.tensor_tensor(
                    out=nr[:], in0=ps_n[:], in1=r[:], op=ALU.mult)
                npre = stp.tile([128, KC, BC], F16, tag=f"npre{l}")
                nc.vector.tensor_tensor(
                    out=npre[:], in0=nr[:], in1=gi_n, op=ALU.add)
                n16 = stp.tile([128, KC, BC], F16, tag=f"n16{l}")
                nc.scalar.activation(n16[:], npre[:], ACTF.Tanh)
                d16 = stp.tile([128, KC, BC], F16, tag=f"d16{l}")
                nc.vector.tensor_tensor(
                    out=d16[:], in0=hprev, in1=n16[:], op=ALU.subtract)

                z16 = stp.tile([128, KC, BC], F16, tag=f"z16{l}")
                nc.scalar.activation(z16[:], ps_z[:], ACTF.Sigmoid)
                e16 = stp.tile([128, KC, BC], F16, tag=f"e16{l}")
                nc.vector.tensor_tensor(
                    out=e16[:], in0=d16[:], in1=z16[:], op=ALU.mult)

                if l == 0:
                    hdst = out0[:, :, t * BC:(t + 1) * BC]
                else:
                    hdst = y_stage[:, :, tt * BC:(tt + 1) * BC]
                nc.vector.tensor_tensor(
                    out=hdst, in0=e16[:], in1=n16[:], op=ALU.add)

            SKEW = 2
            f0 = Feeder(0)
            f1 = Feeder(1)
            ops0 = MC * (KC + 1) + KC
            ops1 = MC * (KC + 1)
            pump0 = ops0 // TB + 1
            pump1 = ops1 // TB + 1

            f0.start(0)
            gi0_cur = f0.finish()
            gi1_cur = None
            y_stage = None
            prev_stage = None

            for sb in range(nb + SKEW):
                b0 = sb
                b1 = sb - SKEW
                if b0 < nb and b0 + 1 < nb:
                    f0.start(b0 + 1)
                if sb >= SKEW - 1 and sb - SKEW + 1 < nb:
                    f1.start(sb - SKEW + 1)
                if b1 >= 0:
                    gi1_cur = f1_pending
                if b1 >= 0:
                    y_stage = stp.tile([128, KC, TB * BC], F16, tag="yst")

                for tt in range(TB):
                    if b0 < nb:
                        emit_step(0, b0, tt, gi0_cur, None, None, f0, pump0)
                    if b1 >= 0:
                        emit_step(1, b1, tt, gi1_cur, y_stage, prev_stage,
                                  f1, pump1)

                if b1 >= 0:
                    nc.sync.dma_start(y_out[b1], y_stage[:])
                    prev_stage = y_stage
                if b0 < nb:
                    gi0_cur = f0.finish()
                f1_pending = f1.finish()

            nc.sync.dma_start(
                h_out[0], out0[:, :, (s_len - 1) * BC:s_len * BC])
            nc.sync.dma_start(
                h_out[1], prev_stage[:, :, (TB - 1) * BC:TB * BC])

    _split_multi_waits(nc)
    return nc


# ---------------------------------------------------------------------------
# Host side: shard inputs, run SPMD, assemble full outputs.

_cached = {}


def _get_program():
    if "nc" not in _cached:
        _cached["nc"] = build_program(S)
    return _cached["nc"]


def _prep_x(x_slice):
    """[BC, S, D] fp32 -> [128, KC, S*BC] fp16 (feature-major columns (t,b))."""
    a = x_slice.transpose(2, 1, 0)                 # [D, S, BC]
    a = a.reshape(KC, 128, S * BC)                 # d = 128k + p
    return np.ascontiguousarray(a.transpose(1, 0, 2)).astype(np.float16)


def _prep_w(w):
    """[3H, H] fp32 -> W^T partition-first [128, KC, 3H] fp16."""
    a = w.T.reshape(KC, 128, G3)
    return np.ascontiguousarray(a.transpose(1, 0, 2)).astype(np.float16)


def _prep_pf(v, dtype=np.float32):
    """[X] per-feature vector -> [128, X//128] partition-first."""
    return np.ascontiguousarray(v.reshape(-1, 128).T).astype(dtype)


def kernel(input, encoder_h, Wih_f, Whh_f, bih_f, bhh_f,
           Wih_b, Whh_b, bih_b, bhh_b):
    input = np.asarray(input, np.float32)
    encoder_h = np.asarray(encoder_h, np.float32)
    nc = _get_program()

    x_rev = input[:, ::-1, :]
    in_maps = []
    for c in range(NCORES):
        fwd = c < 4
        bs = slice((c % 4) * BC, (c % 4) * BC + BC)
        Wih = np.asarray(Wih_f if fwd else Wih_b, np.float32)
        Whh = np.asarray(Whh_f if fwd else Whh_b, np.float32)
        bih = np.asarray(bih_f if fwd else bih_b, np.float32)
        bhh = np.asarray(bhh_f if fwd else bhh_b, np.float32)
        x = input[bs] if fwd else x_rev[bs]

        wih_c = np.stack([_prep_w(Wih[l]) for l in range(L)])
        whh_c = np.stack([_prep_w(Whh[l]) for l in range(L)])
        # r,z gate rows get bih+bhh folded into gi; n rows get bih only
        gb = bih.copy()
        gb[:, :2 * H] += bhh[:, :2 * H]
        gbias_c = np.stack([_prep_pf(gb[l]) for l in range(L)])
        bhhn_b = np.stack([
            np.repeat(_prep_pf(bhh[l, 2 * H:], np.float16)[:, :, None],
                      BC, axis=2)
            for l in range(L)])

        h0 = encoder_h[:, bs, :H] if fwd else encoder_h[:, bs, H:]  # [L,BC,H]
        h0T = np.ascontiguousarray(
            h0.transpose(0, 2, 1).reshape(L, KC, 128, BC).transpose(0, 2, 1, 3)
        ).astype(np.float32)

        in_maps.append({
            "xT": _prep_x(x),
            "wih": wih_c, "whh": whh_c,
            "gbias": gbias_c.astype(np.float32),
            "ident": np.eye(128, dtype=np.float16),
            "bhhnb": np.ascontiguousarray(bhhn_b),
            "h0T": h0T,
        })

    res = run_bass_kernel_spmd(nc, in_maps, core_ids=list(range(NCORES)))

    out = np.empty((B, S, 2 * H), np.float32)
    h = np.empty((L, B, 2 * H), np.float32)
    nb = S // TB
    for c in range(NCORES):
        fwd = c < 4
        bs = slice((c % 4) * BC, (c % 4) * BC + BC)
        y = res.results[c]["y_out"].astype(np.float32)
        hf = res.results[c]["h_out"].astype(np.float32)
        # y: [nb, 128, KC, TB*BC] -> [BC, S, H]
        y = y.reshape(nb, 128, KC, TB, BC)
        y = y.transpose(4, 0, 3, 2, 1).reshape(BC, S, H)
        if not fwd:
            y = y[:, ::-1, :]
        col = slice(0, H) if fwd else slice(H, 2 * H)
        out[bs, :, col] = y
        # hf: [L, 128, KC, BC] -> [L, BC, H]
        hh = hf.transpose(0, 3, 2, 1).reshape(L, BC, H)
        h[:, bs, col] = hh
    return out, h
